# revision 26
# baseline (speedup 1.0000x reference)
"""Longformer sliding-window self-attention on 8 Trainium2 NeuronCores.

Problem: B=2, S=4096, E=768, H=12 heads, D=64, one-sided window W=256.
Sharding: batch*head parallel - core i handles batch i//4, heads 3*(i%4)..+3.
Each core is fully independent (no collectives).

Changes over the v1 baseline (bf16 everywhere; 127.0us -> 114.2us measured):
 - QK score matmuls are zero-padded to K=128: lhsT is the FULL 128-partition
   kq group tile ([k_h0|k_h1] or [k2|q2]) and rhs is a zero-padded q copy
   (qp) whose other 64 partitions are 0, so the extra contraction rows
   contribute exactly 0. Every matmul in the kernel now shares the
   (128,128) weight geometry -> the ~100ns half-speed penalty the PE pays
   on each weight-geometry switch (K=64 <-> K=128, ~146 occurrences,
   ~12us) vanishes. qp data halves are filled by 3 tiny SBUF->SBUF DMAs
   per chunk (replacing the old q2b partition-rebase copy); zero halves
   are memset once and never rewritten.
 - No on-device softmax normalization: PV output (64 cols + denominator
   column from the ones-column-in-V trick) is copied psum->SBUF (DVE,
   ~290ns/task, replacing reciprocal+2 tensor_scalar ~740ns/task) and
   DMA'd to DRAM fully contiguous ([task, 128, 132] f32); the host does
   the divide during unsharding. Kills ~34us of DVE work and the
   strided-DMA drain tail at the end of the run.
 - Masking moved GpSimd -> DVE as ONE strided TT per task over a
   [qh-dup][kind] mask layout (~300ns vs 2x ~620ns); late GpSimd masks
   were stalling PV et-LDWEIGHTS (~60 stretched PV matmuls).
 - Warm-up dummy-tile memsets are the first GpSimd-queue ops so the 36
   p-state warm-up matmuls start at ~6.9us (they were sliding to ~11us).
 - v_sb memset covers only the ones column (was: whole tile, 5.3us DVE).
 - PV psum parity widened to 3 (ps_o [128,3,2,66], one bank) so the out
   DMA has ~3 task periods to drain before WAR reuse; V-projection pieces
   sit between the kq projection groups to cover psq's single-buffer WAR.

Tried and rejected (measured): fp8 QK/projections (rel err 1.4e-2/2.2e-2
vs the 2e-2 gate), row-tiled K=64 head-paired QK with fp8 weights (the
Tile scheduler statically groups each head's matmuls by psum-buffer
readiness so the concurrent pairs never form; +13us), spreading startup
DMAs across Scalar/GpSimd queues (+3..7us), kq bias-copies on DVE (+7us).

Per-core device program (SPMD, identical on all 8 cores):
  inputs (host-prepared, bf16 unless noted):
    hT6   [128, 8, 6, 512] hidden[b].T chunk-major (contiguous DMAs)
    wqk3  [128, 2304]     3 proj groups x 6 ki x 128 cols, q pre-scaled
    wv    [128, 1152]     6 ki x 192 cols (Wv for the 3 heads)
    masks [128, 256]      0/1 lo|hi triangular masks [128,2,128]
    bqk   [128, 3]        f32 per-group per-partition bias
    bvb   [128, 192]      f32 bv broadcast (folded into V projection)
  output:
    out   [48, 128, 132]  f32 unnormalized per-task PV output
                          [m*3+h][query p][qh*66 + (d|denom|pad)]

  Projections: kT/qT in transposed [d, S] layout via lhsT=wqk3-group
  rhs=hT-chunk matmuls (ap=512); v in natural [S, d] layout via
  lhsT=hT-subtile rhs=wv (ap=192), psum->SBUF add of bvb on DVE. kq
  psum -> SBUF bf16 copy with per-partition bias on ACT, then qp DMAs.

  Attention task i = (block m, head h) = query tiles t=2m,2m+1:
  scores^T [key, query] for 2x5 key-tile slots -> one [128,1280] psum;
  one Exp -> et bf16; 0/1 mask multiply on the extreme slots (GpSimd);
  PV with et as the STATIONARY operand: out [128 q, 65] lands directly
  in natural [query, d|denominator] layout; tail: DVE copy of
  ps_o[:,par] -> SBUF, one contiguous [128,132] f32 DMA per task.
"""

import numpy as np
from ml_dtypes import bfloat16

import concourse.bass as bass
import concourse.bacc as bacc
import concourse.mybir as mybir
import concourse.tile as tile
from concourse.bass_utils import run_bass_kernel_spmd

B, S, E, H, D, W = 2, 4096, 768, 12, 64, 256
NCORES = 8
HPC = 3  # heads per core
QB = 256  # queries per attention block
NBLK = S // QB  # 16
KI = E // 128  # 6 contraction tiles
PCH = 512  # projection N-chunk (along S)
NCH = S // PCH  # 8
F32 = mybir.dt.float32
BF16 = mybir.dt.bfloat16
F8 = mybir.dt.float8e4
NEG = -1e30
Act = mybir.ActivationFunctionType
Alu = mybir.AluOpType
NTASK = NBLK * HPC  # 48
PAR = 3  # ps_o parity depth


def _mask_np():
    """Fine 0/1 masks [2][128][128]: lo (valid iff t' >= p), hi (t' <= p).

    t' = key offset within a 128 key tile (partition dim), p = query
    offset within a 128-query tile (free dim). Applied multiplicatively
    to the exp'd probs (et) on GpSimd: the extreme key tiles (kappa = t-2
    and t+2) of each query tile t are triangular; the middle 3 are fully
    valid."""
    p = np.arange(128)[None, :]
    t = np.arange(128)[:, None]
    lo = np.where(t >= p, 1.0, 0.0)
    hi = np.where(t <= p, 1.0, 0.0)
    return np.stack([lo, hi]).astype(np.float32)


def _qtile_plan(t):
    """(lo, hi) inclusive key-tile range for 128-query tile t; the slot of
    key tile kappa within the 5-slot window is kappa - (t-2)."""
    return max(0, t - 2), min(S // 128 - 1, t + 2)


def _build_nc():
    nc = bacc.Bacc()
    # hT6 layout [p][chunk][ki][s]: per-partition contiguous 6KB per chunk
    ht_d = nc.declare_dram_parameter("hT6", [128, NCH, KI, PCH], BF16, isOutput=False)
    wqk_d = nc.declare_dram_parameter("wqk3", [128, HPC * KI * 128], BF16, isOutput=False)
    wv_d = nc.declare_dram_parameter("wv", [128, KI * 192], BF16, isOutput=False)
    msk_d = nc.declare_dram_parameter("masks", [128, 2 * 2 * 128], BF16, isOutput=False)
    bqk_d = nc.declare_dram_parameter("bqk", [128, HPC], F32, isOutput=False)
    bvb_d = nc.declare_dram_parameter("bvb", [128, 192], F32, isOutput=False)
    out_d = nc.declare_dram_parameter("out", [NTASK, 128, 2 * 66], F32, isOutput=True)

    with tile.TileContext(nc) as tc:
        with (
            tc.tile_pool(name="const", bufs=1) as const,
            tc.tile_pool(name="hpool", bufs=3) as hpool,
            tc.tile_pool(name="work", bufs=2) as work,
            tc.tile_pool(name="outp", bufs=4) as outp,
            tc.tile_pool(name="ps", bufs=1, space="PSUM") as psp,
        ):
            # ---- persistent tiles (DMA order: critical-path first) ----
            # warm-up dummy tiles FIRST: their memsets must be the first ops
            # on the GpSimd queue (before any DMA issues land anywhere) so
            # the PE p-state warm-up matmuls below start at ~6.5us, not 11us
            dum_w = const.tile([128, 128], BF16)
            nc.gpsimd.memset(dum_w, 0.0)
            dum_r = const.tile([128, 132], BF16)
            nc.gpsimd.memset(dum_r, 0.0)

            # startup DMAs all on SP: spreading them to Scalar/GpSimd was
            # tried and regressed (ACT table load + SWDGE gen delayed the
            # transfers past the warm-up window).
            wqk = const.tile([128, HPC, KI, 128], BF16)
            nc.sync.dma_start(wqk[:, 0], wqk_d[:, 0 : KI * 128])
            # chunk 0 hT in two TILES (not one tile, two DMAs): reads wait
            # the last write to a tile, so ki<2 matmuls start ~1us earlier.
            # (A 3-way split with wqk groups 1:3 hoisted between the pieces
            # was tried: regressed ~3us.)
            hts0a = hpool.tile([128, 2, PCH], BF16, tag="ht0a", bufs=1)
            nc.sync.dma_start(hts0a, ht_d[:, 0, 0:2])
            hts0b = hpool.tile([128, KI - 2, PCH], BF16, tag="ht0b", bufs=1)
            nc.sync.dma_start(hts0b, ht_d[:, 0, 2:KI])
            nc.sync.dma_start(wqk[:, 1:3], wqk_d[:, KI * 128 :])
            bqk = const.tile([128, HPC], F32)
            nc.sync.dma_start(bqk, bqk_d[:])
            wv = const.tile([128, KI, 192], BF16)
            nc.sync.dma_start(wv, wv_d[:])
            bvb = const.tile([128, 192], F32)
            nc.sync.dma_start(bvb, bvb_d[:])
            # masks layout [p][qh-dup][kind lo|hi][128] so one strided TT per
            # task masks both query tiles' extreme slots
            masks = const.tile([128, 2, 2, 128], BF16)

            # transposed [d, S] projections: G0=[k0|k1], G1=[q0|q1], G2=[k2|q2].
            # per-chunk tiles: dependency tracking is tile-granular, so one
            # big kq tile would make every QK matmul wait on the projection
            # copy most recently emitted before it, not just its own chunks
            kq = [
                const.tile([128, HPC, PCH], BF16, name=f"kq{c}") for c in range(NCH)
            ]
            # zero-padded q copies for K=128 QK matmuls: qp[c][:, h, :] has
            # head h's qT in its home 64 partitions and 0 in the other 64,
            # so lhsT can be the full kq group tile (the other head's k rows
            # hit zeros). Data halves DMA'd per chunk; zero halves memset
            # once below and never rewritten. (Row-tiled K=64 head-pairing
            # with fp8 weights was tried: the Tile scheduler groups each
            # head's matmuls by psum-buffer readiness, so the concurrent
            # pairs never form and switch penalties return -- net loss.)
            qp = [
                const.tile([128, HPC, PCH], BF16, name=f"qp{c}") for c in range(NCH)
            ]
            for c in range(NCH):
                nc.vector.memset(qp[c], 0.0)

            def _ck(sl):
                c, off = divmod(sl.start, PCH)
                return c, slice(off, off + (sl.stop - sl.start))

            def kT_ap(h, sl):
                # full 128-partition group tile (K=128 zero-padded QK)
                c, s = _ck(sl)
                return kq[c][:, (0, 0, 2)[h], s]

            def qT_ap(h, sl):
                c, s = _ck(sl)
                return qp[c][:, h, s]

            # v in natural [S, d] layout: [s-tile-of-128, head, key-tile, d+ones].
            # Only the ones column (softmax denominator) needs initialization;
            # the V-projection TTs write cols 0:64.
            v_sb = const.tile([128, HPC, S // 128, D + 1], BF16)
            nc.vector.memset(v_sb[:, :, :, D : D + 1], 1.0)

            # PV out psum in natural [query, d+denom] layout, [parity][qh][66]
            ps_o = psp.tile([128, PAR, 2, 66], F32, tag="po")

            # PE p-state warm-up: the PE is idle from the end of the engine
            # preamble (~6.5us) until the first input DMA lands (~9us), and
            # the clock ramp (0.65 -> 2.4 GHz) needs ~3us of continuous
            # execution. The dummy matmuls (tiles memset at the very top)
            # run immediately and the real matmuls queue behind them already
            # at full clock. Results land in ps_o and are discarded (first
            # real PV write uses start=True).
            for _ in range(36):
                nc.tensor.matmul(
                    ps_o[:, 0, :, :], dum_w, dum_r, start=True, stop=True
                )

            # ---- Phase B emitters: projection pieces (for fine interleave) ----
            def emit_ht_dma(c):
                # returns an accessor ki -> [128, PCH] AP
                if c == 0:
                    return lambda ki: (
                        hts0a[:, ki, :] if ki < 2 else hts0b[:, ki - 2, :]
                    )
                hts = hpool.tile([128, KI, PCH], BF16, tag="ht", name="hts")
                nc.sync.dma_start(hts, ht_d[:, c])
                return lambda ki: hts[:, ki, :]

            def emit_proj_group(c, g, hts):
                # own 1-bank tag: rotating through "sA" made each psq wait a
                # recycled score-psum's chunky exp on ACT (bufs=2 would fix
                # the ~13 half-speed first-of-group matmuls but PSUM is full:
                # sA 2x3 banks + pq 1 + po 1 = 8)
                psq = psp.tile([128, PCH], F32, tag="pq", bufs=1, name="psq")
                for ki in range(KI):
                    nc.tensor.matmul(
                        psq,
                        wqk[:, g, ki, :],
                        hts(ki),
                        start=(ki == 0),
                        stop=(ki == KI - 1),
                    )
                # bias-add + bf16 cast on ACT (a DVE tensor_scalar here was
                # tried and regressed ~7us: it queues behind mask TTs/out
                # copies and delays the qp DMAs that gate QK)
                nc.scalar.activation(
                    kq[c][:, g, :],
                    psq,
                    Act.Identity,
                    bias=bqk[:, g : g + 1],
                    scale=1.0,
                )
                if g == 1:
                    # q0 (home partitions 0:64), q1 (home 64:128)
                    nc.sync.dma_start(qp[c][0:64, 0, :], kq[c][0:64, 1, :])
                    nc.sync.dma_start(qp[c][64:128, 1, :], kq[c][64:128, 1, :])
                if g == 2:
                    # q2 rebased from partitions 64:128 down to 0:64
                    nc.sync.dma_start(qp[c][0:64, 2, :], kq[c][64:128, 2, :])
                if c == 0 and g == 2:
                    # needed only by attention; issued behind the hot-path DMAs
                    nc.sync.dma_start(masks, msk_d[:])

            def emit_v_mms(c, js, hts, psv):
                # all matmuls before any DVE copy-out: dependency tracking is
                # tile-granular, so a TT between groups stalls the next group
                for j in js:
                    for ki in range(KI):
                        nc.tensor.matmul(
                            psv[:, j, 0:192],
                            hts(ki)[:, j * 128 : (j + 1) * 128],
                            wv[:, ki, :],
                            start=(ki == 0),
                            stop=(ki == KI - 1),
                        )

            def emit_v_tts(c, psv):
                for j in range(PCH // 128):
                    g = (PCH // 128) * c + j
                    nc.vector.tensor_tensor(
                        v_sb[:, :, g, 0:D],
                        psv[:, j, 0:192].rearrange("p (h d) -> p h d", h=HPC),
                        bvb.rearrange("p (h d) -> p h d", h=HPC),
                        Alu.add,
                    )

            def emit_chunk_pieces(c):
                hts = emit_ht_dma(c)
                pieces = [lambda g=g: emit_proj_group(c, g, hts) for g in range(HPC)]
                psv = [None]

                def vmms(js):
                    if psv[0] is None:
                        psv[0] = psp.tile(
                            [128, PCH // 128, 256], F32, tag="sA", bufs=2, name="psv"
                        )
                    emit_v_mms(c, js, hts, psv[0])

                # V pieces BETWEEN the kq groups: psq is single-buffered, so
                # group g+1's first matmul otherwise stalls on group g's ACT
                # bias-copy drain (13 half-speed psq matmuls); the V matmuls
                # (different psum tag) cover that drain.
                pieces = [
                    pieces[0],
                    lambda: vmms((0, 1)),
                    pieces[1],
                    lambda: (vmms((2, 3)), emit_v_tts(c, psv[0])),
                    pieces[2],
                ]
                return pieces

            # ---- Phase C: attention, software-pipelined tasks ----
            pend = {}

            def emit_qk(i):
                # task i = (block m, head h) covers the two 128-query tiles
                # t = 2m, 2m+1, each with a 5-slot key window (kappa = t-2 ..
                # t+2 clipped); slots live at et/psum cols qh*640 + slot*128.
                # Unused edge slots hold exp(garbage) but are never read.
                m, h = divmod(i, HPC)
                ps_s = psp.tile([128, 1280], F32, tag="sA", bufs=2, name="ps_s")
                for qh in range(2):
                    t = 2 * m + qh
                    lo, hi = _qtile_plan(t)
                    qsl = slice(t * 128, (t + 1) * 128)
                    for ka in range(lo, hi + 1):
                        o = ka - (t - 2)
                        col = qh * 640 + o * 128
                        nc.tensor.matmul(
                            ps_s[:, col : col + 128],
                            kT_ap(h, slice(ka * 128, (ka + 1) * 128)),
                            qT_ap(h, qsl),
                            start=True,
                            stop=True,
                        )
                # exp + extreme-slot masking: both query tiles of the task
                # share has_lo = m>=1 (t>=2) and has_hi = m<=14 (t<=29), so
                # ONE strided DVE TT covers the 2-4 extreme slots (lo at
                # slot 0, hi at slot 4, per qh at stride 640). On DVE (not
                # GpSimd): ~300ns vs ~620ns, and late masks were stalling
                # PV et-LDWEIGHTS.
                et = work.tile([128, 1280], BF16, tag="et", name="et", bufs=4)
                nc.scalar.activation(et, ps_s, Act.Exp)
                ev = et.rearrange("p (q s c) -> p q s c", q=2, c=128)
                has_lo, has_hi = m >= 1, m <= NBLK - 2
                if has_lo and has_hi:
                    ap = ev[:, :, 0:5:4, :]
                    mk = masks
                elif has_hi:
                    ap = ev[:, :, 4, :]
                    mk = masks[:, :, 1, :]
                else:
                    ap = ev[:, :, 0, :]
                    mk = masks[:, :, 0, :]
                nc.vector.tensor_tensor(ap, ap, mk, Alu.mult)
                pend[i] = (m, h, et)

            def emit_pv(i):
                # et as the stationary operand: out [128 queries, 65] lands
                # directly in natural [q, d|denom] layout - no PE transposes,
                # no psum->SBUF cast, and PV shares the (128,128) weight
                # geometry with everything else. bf16 LDWEIGHTS (~30ns) hides
                # under the matmuls.
                m, h, et = pend[i]
                par = i % PAR
                for qh in range(2):
                    t = 2 * m + qh
                    lo, hi = _qtile_plan(t)
                    po = ps_o[:, par, qh, 0 : D + 1]
                    for ka in range(lo, hi + 1):
                        o = ka - (t - 2)
                        col = qh * 640 + o * 128
                        nc.tensor.matmul(
                            po,
                            et[:, col : col + 128],
                            v_sb[:, h, ka, :],
                            start=(ka == lo),
                            stop=(ka == hi),
                        )

            def emit_out(i):
                # unnormalized [q, (d|denom)x2] -> SBUF (DVE) -> contiguous
                # DRAM; the host divides by the denominator while unsharding
                m, h, et = pend.pop(i)
                par = i % PAR
                ob = outp.tile([128, 2 * 66], F32, tag="ob", name="ob")
                nc.vector.tensor_scalar(
                    ob, ps_o[:, par, :, :], 0.0, None, Alu.add
                )
                # last few DMAs issue from the then-idle ACT queue so the
                # final drain isn't serialized on SP's ~0.65us/issue
                if i >= NTASK - 4:
                    nc.scalar.dma_start(out_d[i], ob)
                else:
                    nc.sync.dma_start(out_d[i], ob)

            # ---- unified emission: interleave projection pieces with the
            # attention tasks they unblock (block m needs chunks <= (m+1)//2),
            # one piece between consecutive tasks so ACT/DVE attention work
            # spreads across the whole run and only the PE paces. Attention
            # tasks stay software-pipelined: QK(i), PV(i-1), out(i-2).
            i = 0

            credit = [0]

            def pump(n, flush=False):
                # Process tasks in PAIRS with same-kind ops adjacent (2 outs,
                # 2 QKs, 2 PVs). Credits accumulate across calls so pairs
                # actually form. PV lags QK by 2 tasks so et (exp+mask) is
                # always ready; outs lag by 4.
                nonlocal i
                credit[0] += n
                while credit[0] >= 2 or (flush and credit[0] > 0):
                    k = min(2, credit[0])
                    for d in range(k):
                        if 0 <= i + d - 4 < NTASK:
                            emit_out(i + d - 4)
                    for d in range(k):
                        if i + d < NTASK:
                            emit_qk(i + d)
                    for d in range(k):
                        if 0 <= i + d - 2 < NTASK:
                            emit_pv(i + d - 2)
                    i += k
                    credit[0] -= k

            # tasks of block m are emitted only after every piece of chunk
            # K(m) = min((m+1)//2, NCH-1): reads must follow their producing
            # writes in program order for Tile to insert the dependency.
            for c in range(NCH):
                pieces = emit_chunk_pieces(c)
                avail = HPC * len(
                    [m for m in range(NBLK) if min((m + 1) // 2, NCH - 1) < c]
                )
                base = i
                for pi, piece in enumerate(pieces):
                    piece()
                    budget = avail - base
                    share = budget * (pi + 1) // len(pieces) - budget * pi // len(
                        pieces
                    )
                    pump(min(share, avail - i - credit[0]))
            pump(NTASK + 4 - i - credit[0], flush=True)
    nc.compile()
    return nc


_CACHE = {}


def _get_nc():
    if "nc" not in _CACHE:
        _CACHE["nc"] = _build_nc()
    return _CACHE["nc"]


def make_in_maps(hidden_states, Wq, bq, Wk, bk, Wv, bv):
    hidden_states = np.asarray(hidden_states, dtype=np.float32)
    Wq = np.asarray(Wq, dtype=np.float32)
    Wk = np.asarray(Wk, dtype=np.float32)
    Wv = np.asarray(Wv, dtype=np.float32)
    bq = np.asarray(bq, dtype=np.float32)
    bk = np.asarray(bk, dtype=np.float32)
    bv = np.asarray(bv, dtype=np.float32)
    scale = 1.0 / float(np.sqrt(D))
    # masks layout [p][qh-dup][kind][128]: duplicated over the qh axis so a
    # single strided TT masks both query tiles of a task
    mpkc = _mask_np().transpose(1, 0, 2)  # [p][kind][c]
    masks = np.ascontiguousarray(
        np.broadcast_to(mpkc[:, None], (128, 2, 2, 128)).reshape(128, 512)
    ).astype(bfloat16)
    in_maps = []
    for core in range(NCORES):
        b = core // (NCORES // B)
        h0 = HPC * (core % (NCORES // B))
        hsl = [slice(D * (h0 + hh), D * (h0 + hh + 1)) for hh in range(HPC)]
        cols = slice(D * h0, D * (h0 + HPC))
        G = np.empty((HPC, E, 128), np.float32)
        G[0][:, 0:D] = Wk[:, hsl[0]]
        G[0][:, D:128] = Wk[:, hsl[1]]
        G[1][:, 0:D] = Wq[:, hsl[0]] * scale
        G[1][:, D:128] = Wq[:, hsl[1]] * scale
        G[2][:, 0:D] = Wk[:, hsl[2]]
        G[2][:, D:128] = Wq[:, hsl[2]] * scale
        # wqk3[p, g, ki, m] = G[g][ki*128+p, m]
        wqk3 = G.reshape(HPC, KI, 128, 128).transpose(2, 0, 1, 3)
        bqk = np.empty((128, HPC), np.float32)
        bqk[0:D, 0] = bk[hsl[0]]
        bqk[D:128, 0] = bk[hsl[1]]
        bqk[0:D, 1] = bq[hsl[0]] * scale
        bqk[D:128, 1] = bq[hsl[1]] * scale
        bqk[0:D, 2] = bk[hsl[2]]
        bqk[D:128, 2] = bq[hsl[2]] * scale
        # wv[p, ki, n] = Wv[ki*128+p, cols[n]]
        wv_p = Wv[:, cols].reshape(KI, 128, HPC * D).transpose(1, 0, 2)
        bvb = np.broadcast_to(bv[cols], (128, HPC * D)).copy()
        # hT6[p, c, ki, s] = hidden[b].T[ki*128+p, c*512+s]
        hT6 = np.ascontiguousarray(
            hidden_states[b].T.reshape(KI, 128, NCH, PCH).transpose(1, 2, 0, 3)
        ).astype(bfloat16)
        in_maps.append(
            dict(
                hT6=hT6,
                wqk3=np.ascontiguousarray(wqk3).reshape(128, HPC * KI * 128).astype(bfloat16),
                wv=np.ascontiguousarray(wv_p).reshape(128, KI * 192).astype(bfloat16),
                masks=np.ascontiguousarray(masks),
                bqk=bqk,
                bvb=bvb,
            )
        )
    return in_maps


def kernel(hidden_states, Wq, bq, Wk, bk, Wv, bv):
    in_maps = make_in_maps(hidden_states, Wq, bq, Wk, bk, Wv, bv)
    res = run_bass_kernel_spmd(_get_nc(), in_maps, list(range(NCORES)))
    kernel.last = res
    out = np.empty((B, S, E), np.float32)
    for core in range(NCORES):
        r = res.results[core]["out"]  # [48, 128, 132] unnormalized
        b = core // (NCORES // B)
        h0 = HPC * (core % (NCORES // B))
        rr = r.reshape(NBLK, HPC, 128, 2, 66)
        o = rr[..., 0:D] / rr[..., D : D + 1]  # [m, h, p, qh, d]
        # queries: (m, qh, p) -> m*256 + qh*128 + p; cols: (h, d)
        o = o.transpose(0, 3, 2, 1, 4).reshape(S, HPC * D)
        out[b, :, D * h0 : D * (h0 + HPC)] = o
    return out
